# revision 14
# baseline (speedup 1.0000x reference)
"""GraphWeightedMHA on 8 trn2 cores — head-sharded bf16 Bass/Tile kernel.

Sharding: one attention head per core (tensor parallel); sgconv + final
projection row-sharded. No k/v collective; only the per-head [4096, 64]
attention output is AllGathered (two overlapped halves, in transposed
[hd, seq] layout so the sgconv-side reads use the hardware DMA transpose).

Attention runs entirely as 64-row PE tiles alternating row groups, so
weight loads always overlap a streaming matmul on the other half of the
array: S^T as row-packed pairs (keys ktA on rows 0-63 from the swapped
[kT_lo;qT_hi] copy, ktB on rows 64-127), and PV split into key-halves
(rows 0-63 -> accumulator o_A, rows 64-127 -> o_B; summed during the
normalize). V is ones-augmented (col 64) for the softmax denominators and
zero-padded to 128 cols so FWL engages. exp is split between the scalar
engine (exact, 9/16 pairs) and a DVE Schraudolph fast-exp. The PE stream is
software-pipelined: S^T pairs run LOOK ahead of their PV consumers.

All matmuls bf16 (fp32 PSUM accumulation) — fp32r would compile to the ~3x
slower fp32_mode=HIGH multi-pass path.
"""
import numpy as np
import ml_dtypes
import concourse.bass as bass
import concourse.bacc as bacc
import concourse.tile as tile
from concourse import mybir
from concourse.bass_utils import run_bass_kernel_spmd

dt = mybir.dt
bf16 = ml_dtypes.bfloat16
NC = 8
N, D, H, HD = 4096, 512, 8, 64
RS = N // NC          # 512 output rows per core for sgconv/final proj
NB = D // 128         # 4 blocks of 128 along D
QB = 8                # query blocks of 512
KT = N // 128         # 32 key tiles
NPAIR = KT // 2       # 16 S^T pairs per query block
LOOK = 3              # S^T pair lookahead ahead of PV in the PE stream
SCALE = float(1.0 / np.sqrt(np.float32(D)))
Exp = mybir.ActivationFunctionType.Exp
Ident = mybir.ActivationFunctionType.Identity
Mult = mybir.AluOpType.mult
Add = mybir.AluOpType.add

# Schraudolph fast-exp in bf16 bit space: bf16_bits(exp(x)) ~ round(A*x + B)
FEXP_A = float(np.float32(128.0 / np.log(2.0)))
FEXP_B = float(np.float32(127 * 128 - 5.0))
# pairs using the scalar engine's exact exp (rest: DVE fast-exp)
ACT_SET = {0, 2, 4, 6, 8, 10, 12, 13, 15}

_CACHE: dict = {}


def blk(x):  # [512, M] -> [128, 4, M]  (p, kb, m) with d = kb*128+p
    return np.ascontiguousarray(x.reshape(NB, 128, -1).transpose(1, 0, 2))


def _build():
    nc = bacc.Bacc("TRN2", target_bir_lowering=False, debug=False, num_devices=NC)

    def din(name, shape, d=dt.bfloat16):
        return nc.dram_tensor(name, shape, d, kind="ExternalInput").ap()

    qT_d = din("qT", [128, NB, N])        # query^T blocked (shared)
    kT_d = din("kT", [128, NB, N])        # key^T blocked (shared)
    vT_d = din("vT", [128, NB, N])        # value^T blocked (shared)
    wq_d = din("wq", [128, NB, HD])       # (s*Wq_h)^T blocked
    wk_d = din("wk", [128, NB, HD])       # Wk_h^T blocked
    wv_d = din("wv", [128, NB, HD])       # Wv_h^T blocked
    wo_d = din("wo", [128, NB, D])        # Wo^T blocked
    bqk_d = din("bqk", [128, 1], dt.float32)  # [s*bq_h ; bk_h]
    bv_d = din("bv", [1, HD])
    bo_d = din("bo", [1, D])
    ones_d = din("ones", [128, 128])
    eye_d = din("eye", [128, 128], dt.float32)
    sgT_d = din("sgT", [N, RS])           # sgconv_mat[rows,:].T per core
    out_d = nc.dram_tensor("out", [RS, D], dt.float32, kind="ExternalOutput").ap()

    with tile.TileContext(nc) as tc:
        with tc.tile_pool(name="const", bufs=1) as cp, \
             tc.tile_pool(name="persist", bufs=1) as pp, \
             tc.tile_pool(name="dram", bufs=1, space="DRAM") as dp:
            wq_sb = cp.tile([128, NB, HD], dt.bfloat16)
            wk_sb = cp.tile([128, NB, HD], dt.bfloat16)
            wv_sb = cp.tile([128, NB, HD], dt.bfloat16)
            wo_sb = cp.tile([128, NB, D], dt.bfloat16)
            bqk_sb = cp.tile([128, 1], dt.float32)
            bv_sb = cp.tile([1, HD], dt.bfloat16)
            bo_sb = cp.tile([1, D], dt.bfloat16)
            ones_sb = cp.tile([128, 128], dt.bfloat16)
            eye_sb = cp.tile([128, 128], dt.float32)
            for sb_t, d_t in [(wq_sb, wq_d), (wk_sb, wk_d), (wv_sb, wv_d),
                              (wo_sb, wo_d), (bqk_sb, bqk_d), (bv_sb, bv_d),
                              (bo_sb, bo_d), (ones_sb, ones_d), (eye_sb, eye_d)]:
                nc.sync.dma_start(sb_t[:], d_t[:])

            sgb = pp.tile([128, KT, RS], dt.bfloat16)      # [j%128, jt, i]

            bounce_a = dp.tile([HD, 2048], dt.bfloat16)  # attn^T blocks 0-3
            bounce_b = dp.tile([HD, 1536], dt.bfloat16)  # blocks 4-6
            bounce_c = dp.tile([HD, 512], dt.bfloat16)   # block 7
            gath_a = dp.tile([NC * HD, 2048], dt.bfloat16, addr_space="Shared")
            gath_b = dp.tile([NC * HD, 1536], dt.bfloat16, addr_space="Shared")
            gath_c = dp.tile([NC * HD, 512], dt.bfloat16, addr_space="Shared")

            with tc.tile_pool(name="attn_state", bufs=1) as ap:
                kT_sb = ap.tile([128, NB, N], dt.bfloat16)
                vT_sb = ap.tile([128, NB, N], dt.bfloat16)
                qT_sb = ap.tile([128, NB, N], dt.bfloat16)
                T1 = ap.tile([128, N], dt.bfloat16)    # [qT_lo ; kT_hi]
                T2 = ap.tile([128, N], dt.bfloat16)    # [kT_lo ; qT_hi]
                vh = ap.tile([128, KT, 128], dt.bfloat16)  # [key, kt, hd|1|0pad]
                attnT_sb = ap.tile([HD, N], dt.bfloat16)   # [hd, seq]
                scr = ap.tile([1, 16], dt.float32)

                # input prefetch, ordered by first use: qk proj -> v -> q tail
                nc.sync.dma_start(kT_sb[:, :, 0:1024], kT_d[:, :, 0:1024])
                nc.sync.dma_start(qT_sb[:, :, 0:1024], qT_d[:, :, 0:1024])
                nc.sync.dma_start(vT_sb[:, :, 0:1024], vT_d[:, :, 0:1024])
                for c in range(1, 4):
                    sl = slice(c * 1024, (c + 1) * 1024)
                    nc.sync.dma_start(kT_sb[:, :, sl], kT_d[:, :, sl])
                    nc.sync.dma_start(vT_sb[:, :, sl], vT_d[:, :, sl])
                for c in range(1, 4):
                    sl = slice(c * 1024, (c + 1) * 1024)
                    nc.sync.dma_start(qT_sb[:, :, sl], qT_d[:, :, sl])
                nc.vector.memset(vh[:, :, HD:128], 0.0)
                nc.vector.memset(vh[:, :, HD:HD + 1], 1.0)
                # preload the exp table set on ACT while DMAs run
                nc.scalar.activation(scr[:], eye_sb[0:1, 0:16], Exp)

                # -------- Phase A: q/k (col-tiled) + v proj, interleaved ---
                # per input chunk: 2 qk blocks + 8 v tiles, so the PE keeps
                # a dense stream while DMA-paced (HAM stays warm).
                with tc.tile_pool(name="pa_ps", bufs=2, space="PSUM") as pa_ps, \
                     tc.tile_pool(name="pv_ps", bufs=3, space="PSUM") as pv_ps:
                    for c in range(4):
                        for nb in (2 * c, 2 * c + 1):
                            sl = slice(nb * 512, (nb + 1) * 512)
                            ps = pa_ps.tile([128, 512], dt.float32, tag="pa")
                            for kb in range(NB):
                                nc.tensor.matmul(ps[0:64, :], wq_sb[:, kb, :],
                                                 qT_sb[:, kb, sl],
                                                 start=(kb == 0),
                                                 stop=(kb == NB - 1),
                                                 tile_position=(0, 0))
                                nc.tensor.matmul(ps[64:128, :], wk_sb[:, kb, :],
                                                 kT_sb[:, kb, sl],
                                                 start=(kb == 0),
                                                 stop=(kb == NB - 1),
                                                 tile_position=(0, 64),
                                                 skip_group_check=True)
                            nc.scalar.activation(T1[:, sl], ps[:], Ident,
                                                 bias=bqk_sb[:])
                            nc.sync.dma_start(T2[0:64, sl], T1[64:128, sl])
                            nc.sync.dma_start(T2[64:128, sl], T1[0:64, sl])
                        for nt in range(8 * c, 8 * c + 8):
                            psv = pv_ps.tile([128, HD], dt.float32, tag="pv")
                            for kb in range(NB):
                                nc.tensor.matmul(
                                    psv[:],
                                    vT_sb[:, kb, nt * 128:(nt + 1) * 128],
                                    wv_sb[:, kb, :], start=(kb == 0), stop=False)
                            nc.tensor.matmul(psv[:], ones_sb[0:1, :], bv_sb[:],
                                             start=False, stop=True)
                            nc.vector.tensor_copy(vh[:, nt, 0:HD], psv[:])

                # sgconv matrix load (late: input DMAs get early bandwidth)
                nc.sync.dma_start(
                    sgb[:], sgT_d[:].rearrange("(jt p) i -> p jt i", jt=KT))

                # ---------------- Phase B: attention ----------------
                with tc.tile_pool(name="s_ps", bufs=3, space="PSUM") as s_pool, \
                     tc.tile_pool(name="o_ps", bufs=2, space="PSUM") as o_pool, \
                     tc.tile_pool(name="pt", bufs=3) as pt_pool, \
                     tc.tile_pool(name="os", bufs=2) as os_pool, \
                     tc.tile_pool(name="rc", bufs=2) as rc_pool:

                    def emit_S(qsl, g):
                        ktA, ktB = 2 * g, 2 * g + 1
                        sps = s_pool.tile([128, 1024], dt.float32, tag="sps")
                        nc.tensor.matmul(
                            sps[:, 0:512], T2[0:64, ktA * 128:(ktA + 1) * 128],
                            T1[0:64, qsl], start=True, stop=True,
                            tile_position=(0, 0))
                        nc.tensor.matmul(
                            sps[:, 512:1024],
                            T1[64:128, ktB * 128:(ktB + 1) * 128],
                            T2[64:128, qsl], start=True, stop=True,
                            tile_position=(64, 0), skip_group_check=True)
                        return sps

                    CHUNKS = [[0, 1, 2], [3, 4, 5], [6, 7, 8],
                              [9, 10, 11], [12, 13, 14], [15]]
                    tail = [None]   # deferred (qb, oc) for dbb/recip/mult

                    def emit_tail():
                        if tail[0] is None:
                            return
                        qb, oc = tail[0]
                        tail[0] = None
                        qsl = slice(qb * 512, (qb + 1) * 512)
                        # replicate the denominator row across partitions
                        # 0-63 with a K=1 matmul (ones at array row 64)
                        dbb = o_pool.tile([128, 512], dt.float32, tag="o")
                        nc.tensor.matmul(dbb[0:HD, :], ones_sb[HD:HD + 1, 0:HD],
                                         oc[HD:HD + 1, :], start=True,
                                         stop=True, tile_position=(64, 0),
                                         skip_group_check=True)
                        rc = rc_pool.tile([HD, 512], dt.float32, tag="rc")
                        nc.vector.reciprocal_approx_fast(rc[:], dbb[0:HD, :])
                        nc.vector.tensor_tensor(
                            attnT_sb[:, qsl], oc[0:HD, :], rc[:], Mult)

                    def emit_ag(bounce, gath, n0, n1):
                        nc.sync.dma_start(bounce[:], attnT_sb[:, n0:n1])
                        nc.gpsimd.collective_compute(
                            "AllGather", mybir.AluOpType.bypass,
                            replica_groups=[list(range(NC))],
                            ins=[bounce[:].opt()], outs=[gath[:].opt()])

                    for qb in range(QB):
                        qsl = slice(qb * 512, (qb + 1) * 512)
                        o_ps = o_pool.tile([128, 512], dt.float32, tag="o")
                        store = {g: emit_S(qsl, g) for g in CHUNKS[0]}
                        emit_tail()
                        if qb == 4:
                            emit_ag(bounce_a, gath_a, 0, 2048)
                        elif qb == 7:
                            emit_ag(bounce_b, gath_b, 2048, 3584)
                        for ci, ch in enumerate(CHUNKS):
                            ps_list = []
                            for g in ch:
                                sps = store.pop(g)
                                p = pt_pool.tile([128, 1024], dt.bfloat16,
                                                 tag="pt")
                                if g in ACT_SET:
                                    nc.scalar.activation(p[:], sps[:], Exp)
                                else:
                                    nc.vector.tensor_scalar(
                                        p[:].bitcast(dt.int16), sps[:],
                                        FEXP_A, FEXP_B, Mult, Add)
                                ps_list.append((g, p))
                            if ci + 1 < len(CHUNKS):
                                for g in CHUNKS[ci + 1]:
                                    store[g] = emit_S(qsl, g)
                            for g, p in ps_list:
                                for t in range(2):
                                    kt = 2 * g + t
                                    nc.tensor.matmul(
                                        o_ps[:], vh[:, kt, :],
                                        p[:, t * 512:(t + 1) * 512],
                                        start=(g == 0 and t == 0),
                                        stop=(g == NPAIR - 1 and t == 1),
                                        skip_group_check=True)
                        # free o_ps fast; the rest of the normalize rides
                        # behind the next block's first S-run
                        oc = os_pool.tile([HD + 1, 512], dt.bfloat16, tag="oc")
                        nc.vector.tensor_copy(oc[:], o_ps[0:HD + 1, :])
                        tail[0] = (qb, oc)
                    emit_tail()
                    emit_ag(bounce_c, gath_c, 3584, N)

            # ---------------- Phase C: sgconv (out_sg^T) ----------------
            # gathered attn^T: [512 d, 2048 seq] per half; hardware DMA
            # transpose yields [128 j, 512 d] lhsT tiles directly.
            with tc.tile_pool(name="aj", bufs=4) as aj_pool, \
                 tc.tile_pool(name="og_ps", bufs=1, space="PSUM") as og_pool, \
                 tc.tile_pool(name="pd_sb", bufs=1) as pd_sb_pool:
                og = og_pool.tile([128, NB, RS], dt.float32)
                for jt in range(KT):
                    gath, j = ((gath_a, jt) if jt < 16 else
                               (gath_b, jt - 16) if jt < 28 else
                               (gath_c, jt - 28))
                    if True:
                        aj = aj_pool.tile([128, D], dt.bfloat16, tag="aj")
                        nc.sync.dma_start_transpose(
                            aj[:], gath[:, j * 128:(j + 1) * 128])
                        for db in range(NB):
                            nc.tensor.matmul(
                                og[:, db, :],
                                aj[:, db * 128:(db + 1) * 128],
                                sgb[:, jt, :], start=(jt == 0),
                                stop=(jt == KT - 1), skip_group_check=True)
                # ---------------- Phase D: final projection ----------------
                ogT = pd_sb_pool.tile([128, NB, RS], dt.bfloat16)
                for db in range(NB):
                    nc.vector.tensor_copy(ogT[:, db, :], og[:, db, :])
                with tc.tile_pool(name="pd_ps", bufs=2, space="PSUM") as pd_ps_pool, \
                     tc.tile_pool(name="po_sb", bufs=2) as po_sb_pool:
                    for it in range(NB):
                        ps = pd_ps_pool.tile([128, D], dt.float32, tag="pd")
                        for db in range(NB):
                            nc.tensor.matmul(
                                ps[:], ogT[:, db, it * 128:(it + 1) * 128],
                                wo_sb[:, db, :], start=(db == 0), stop=False)
                        nc.tensor.matmul(ps[:], ones_sb[0:1, :], bo_sb[:],
                                         start=False, stop=True)
                        po = po_sb_pool.tile([128, D], dt.float32, tag="po")
                        nc.vector.tensor_copy(po[:], ps[:])
                        nc.sync.dma_start(out_d[it * 128:(it + 1) * 128, :], po[:])
    nc.compile()
    return nc


def kernel(**inputs):
    query = np.asarray(inputs["query"], dtype=np.float32)
    key = np.asarray(inputs["key"], dtype=np.float32)
    value = np.asarray(inputs["value"], dtype=np.float32)
    Wq, bq = np.asarray(inputs["Wq"], np.float32), np.asarray(inputs["bq"], np.float32)
    Wk, bk = np.asarray(inputs["Wk"], np.float32), np.asarray(inputs["bk"], np.float32)
    Wv, bv = np.asarray(inputs["Wv"], np.float32), np.asarray(inputs["bv"], np.float32)
    Wo, bo = np.asarray(inputs["Wo"], np.float32), np.asarray(inputs["bo"], np.float32)
    sg = np.asarray(inputs["sgconv_mat"], np.float32)[0]   # [N, N]

    if "nc" not in _CACHE:
        _CACHE["nc"] = _build()
    nc = _CACHE["nc"]

    qTb = blk(query[0].T.astype(bf16))
    kTb = blk(key[0].T.astype(bf16))
    vTb = blk(value[0].T.astype(bf16))
    wob = blk(Wo.T.astype(bf16))
    common = {
        "qT": qTb, "kT": kTb, "vT": vTb, "wo": wob,
        "bo": bo.reshape(1, D).astype(bf16),
        "ones": np.ones((128, 128), bf16),
        "eye": np.eye(128, dtype=np.float32),
    }
    in_maps = []
    for c in range(NC):
        hs = slice(c * HD, (c + 1) * HD)
        rs = slice(c * RS, (c + 1) * RS)
        in_maps.append(dict(
            common,
            wq=blk((SCALE * Wq[hs, :]).T.astype(bf16)),
            wk=blk(Wk[hs, :].T.astype(bf16)),
            wv=blk(Wv[hs, :].T.astype(bf16)),
            bqk=np.concatenate([SCALE * bq[hs], bk[hs]]).reshape(128, 1)
                .astype(np.float32),
            bv=bv[hs].reshape(1, HD).astype(bf16),
            sgT=np.ascontiguousarray(sg[rs, :].T).astype(bf16),
        ))
    res = run_bass_kernel_spmd(nc, in_maps, core_ids=list(range(NC)),
                               **_CACHE.get("run_kwargs", {}))
    _CACHE["last_results"] = res
    out = np.concatenate([res.results[c]["out"] for c in range(NC)], axis=0)
    return out.reshape(1, N, D)


# revision 16
# speedup vs baseline: 1.0816x; 1.0816x over previous
"""GraphWeightedMHA on 8 trn2 cores — head-sharded bf16 Bass/Tile kernel.

Sharding: one attention head per core (tensor parallel); sgconv + final
projection row-sharded. No k/v collective; only the per-head [4096, 64]
attention output is AllGathered (two overlapped halves, in transposed
[hd, seq] layout so the sgconv-side reads use the hardware DMA transpose).

Attention runs entirely as 64-row PE tiles alternating row groups, so
weight loads always overlap a streaming matmul on the other half of the
array: S^T as row-packed pairs (keys ktA on rows 0-63 from the swapped
[kT_lo;qT_hi] copy, ktB on rows 64-127), and PV split into key-halves
(rows 0-63 -> accumulator o_A, rows 64-127 -> o_B; summed during the
normalize). V is ones-augmented (col 64) for the softmax denominators and
zero-padded to 128 cols so FWL engages. exp is split between the scalar
engine (exact, 9/16 pairs) and a DVE Schraudolph fast-exp. The PE stream is
software-pipelined: S^T pairs run LOOK ahead of their PV consumers.

All matmuls bf16 (fp32 PSUM accumulation) — fp32r would compile to the ~3x
slower fp32_mode=HIGH multi-pass path.
"""
import numpy as np
import ml_dtypes
import concourse.bass as bass
import concourse.bacc as bacc
import concourse.tile as tile
from concourse import mybir
from concourse.bass_utils import run_bass_kernel_spmd

dt = mybir.dt
bf16 = ml_dtypes.bfloat16
fp8 = ml_dtypes.float8_e4m3fn
NC = 8
N, D, H, HD = 4096, 512, 8, 64
RS = N // NC          # 512 output rows per core for sgconv/final proj
NB = D // 128         # 4 blocks of 128 along D
QB = 8                # query blocks of 512
KT = N // 128         # 32 key tiles
NPAIR = KT // 2       # 16 S^T pairs per query block
LOOK = 3              # S^T pair lookahead ahead of PV in the PE stream
SCALE = float(1.0 / np.sqrt(np.float32(D)))
Exp = mybir.ActivationFunctionType.Exp
Ident = mybir.ActivationFunctionType.Identity
Mult = mybir.AluOpType.mult
Add = mybir.AluOpType.add

# Schraudolph fast-exp in bf16 bit space: bf16_bits(exp(x)) ~ round(A*x + B)
FEXP_A = float(np.float32(128.0 / np.log(2.0)))
FEXP_B = float(np.float32(127 * 128 - 5.0))
# pairs using the scalar engine's exact exp (rest: DVE fast-exp)
ACT_SET = {0, 2, 4, 6, 8, 10, 12, 13, 15}

_CACHE: dict = {}


def blk(x):  # [512, M] -> [128, 4, M]  (p, kb, m) with d = kb*128+p
    return np.ascontiguousarray(x.reshape(NB, 128, -1).transpose(1, 0, 2))


def _build():
    nc = bacc.Bacc("TRN2", target_bir_lowering=False, debug=False, num_devices=NC)

    def din(name, shape, d=dt.bfloat16):
        return nc.dram_tensor(name, shape, d, kind="ExternalInput").ap()

    qT_d = din("qT", [128, NB, N], dt.float8e4)   # query^T blocked (fp8)
    kT_d = din("kT", [128, NB, N], dt.float8e4)   # key^T blocked (fp8)
    vT_d = din("vT", [128, NB, N])        # value^T blocked (shared)
    wq_d = din("wq", [128, NB, HD], dt.float8e4)  # (64*s*Wq_h)^T blocked
    wk_d = din("wk", [128, NB, HD], dt.float8e4)  # (64*Wk_h)^T blocked
    wv_d = din("wv", [128, NB, HD])       # Wv_h^T blocked
    wo_d = din("wo", [128, NB, D])        # Wo^T blocked
    bqk_d = din("bqk", [128, 1], dt.float32)  # [s*bq_h ; bk_h]
    bv_d = din("bv", [1, HD])
    bo_d = din("bo", [1, D])
    ones_d = din("ones", [128, 128])
    eye_d = din("eye", [128, 128], dt.float32)
    sgT_d = din("sgT", [N, RS])           # sgconv_mat[rows,:].T per core
    out_d = nc.dram_tensor("out", [RS, D], dt.float32, kind="ExternalOutput").ap()

    with tile.TileContext(nc) as tc:
        with tc.tile_pool(name="const", bufs=1) as cp, \
             tc.tile_pool(name="persist", bufs=1) as pp, \
             tc.tile_pool(name="dram", bufs=1, space="DRAM") as dp:
            wq_sb = cp.tile([128, NB, HD], dt.float8e4)
            wk_sb = cp.tile([128, NB, HD], dt.float8e4)
            wv_sb = cp.tile([128, NB, HD], dt.bfloat16)
            wo_sb = cp.tile([128, NB, D], dt.bfloat16)
            bqk_sb = cp.tile([128, 1], dt.float32)
            bv_sb = cp.tile([1, HD], dt.bfloat16)
            bo_sb = cp.tile([1, D], dt.bfloat16)
            ones_sb = cp.tile([128, 128], dt.bfloat16)
            eye_sb = cp.tile([128, 128], dt.float32)
            for sb_t, d_t in [(wq_sb, wq_d), (wk_sb, wk_d), (wv_sb, wv_d),
                              (wo_sb, wo_d), (bqk_sb, bqk_d), (bv_sb, bv_d),
                              (bo_sb, bo_d), (ones_sb, ones_d), (eye_sb, eye_d)]:
                nc.sync.dma_start(sb_t[:], d_t[:])

            sgb = pp.tile([128, KT, RS], dt.bfloat16)      # [j%128, jt, i]
            aj_all = pp.tile([128, KT, D], dt.bfloat16)    # transposed gathers

            bounce_a = dp.tile([HD, 2048], dt.bfloat16)  # attn^T blocks 0-3
            bounce_b = dp.tile([HD, 1536], dt.bfloat16)  # blocks 4-6
            bounce_c = dp.tile([HD, 512], dt.bfloat16)   # block 7
            gath_a = dp.tile([NC * HD, 2048], dt.bfloat16, addr_space="Shared")
            gath_b = dp.tile([NC * HD, 1536], dt.bfloat16, addr_space="Shared")
            gath_c = dp.tile([NC * HD, 512], dt.bfloat16, addr_space="Shared")

            with tc.tile_pool(name="attn_state", bufs=1) as ap:
                kT_sb = ap.tile([128, NB, N], dt.float8e4)
                vT_sb = ap.tile([128, NB, N], dt.bfloat16)
                qT_sb = ap.tile([128, NB, N], dt.float8e4)
                T1 = ap.tile([128, N], dt.bfloat16)    # [qT_lo ; kT_hi]
                T2 = ap.tile([128, N], dt.bfloat16)    # [kT_lo ; qT_hi]
                vh = ap.tile([128, KT, 128], dt.bfloat16)  # [key, kt, hd|1|0pad]
                attnT_sb = ap.tile([HD, N], dt.bfloat16)   # [hd, seq]
                scr = ap.tile([1, 16], dt.float32)

                # input prefetch, ordered by first use: qk proj -> v -> q tail
                nc.sync.dma_start(kT_sb[:, :, 0:1024], kT_d[:, :, 0:1024])
                nc.sync.dma_start(qT_sb[:, :, 0:1024], qT_d[:, :, 0:1024])
                nc.sync.dma_start(vT_sb[:, :, 0:1024], vT_d[:, :, 0:1024])
                for c in range(1, 4):
                    sl = slice(c * 1024, (c + 1) * 1024)
                    nc.sync.dma_start(kT_sb[:, :, sl], kT_d[:, :, sl])
                    nc.sync.dma_start(vT_sb[:, :, sl], vT_d[:, :, sl])
                for c in range(1, 4):
                    sl = slice(c * 1024, (c + 1) * 1024)
                    nc.sync.dma_start(qT_sb[:, :, sl], qT_d[:, :, sl])
                nc.vector.memset(vh[:, :, HD:128], 0.0)
                nc.vector.memset(vh[:, :, HD:HD + 1], 1.0)
                # preload the exp table set on ACT while DMAs run
                nc.scalar.activation(scr[:], eye_sb[0:1, 0:16], Exp)

                # -------- Phase A: q/k (col-tiled) + v proj, interleaved ---
                # per input chunk: 2 qk blocks + 8 v tiles, so the PE keeps
                # a dense stream while DMA-paced (HAM stays warm).
                with tc.tile_pool(name="pa_ps", bufs=2, space="PSUM") as pa_ps, \
                     tc.tile_pool(name="pv_ps", bufs=3, space="PSUM") as pv_ps:
                    for c in range(4):
                        for nb in (2 * c, 2 * c + 1):
                            sl = slice(nb * 512, (nb + 1) * 512)
                            ps = pa_ps.tile([128, 512], dt.float32, tag="pa")
                            for kb in range(NB):
                                nc.tensor.matmul(ps[0:64, :], wq_sb[:, kb, :],
                                                 qT_sb[:, kb, sl],
                                                 start=(kb == 0),
                                                 stop=(kb == NB - 1),
                                                 tile_position=(0, 0))
                                nc.tensor.matmul(ps[64:128, :], wk_sb[:, kb, :],
                                                 kT_sb[:, kb, sl],
                                                 start=(kb == 0),
                                                 stop=(kb == NB - 1),
                                                 tile_position=(0, 64),
                                                 skip_group_check=True)
                            nc.scalar.activation(T1[:, sl], ps[:], Ident,
                                                 bias=bqk_sb[:],
                                                 scale=1.0 / 64.0)
                            nc.sync.dma_start(T2[0:64, sl], T1[64:128, sl])
                            nc.sync.dma_start(T2[64:128, sl], T1[0:64, sl])
                        for nt in range(8 * c, 8 * c + 8):
                            psv = pv_ps.tile([128, HD], dt.float32, tag="pv")
                            for kb in range(NB):
                                nc.tensor.matmul(
                                    psv[:],
                                    vT_sb[:, kb, nt * 128:(nt + 1) * 128],
                                    wv_sb[:, kb, :], start=(kb == 0), stop=False)
                            nc.tensor.matmul(psv[:], ones_sb[0:1, :], bv_sb[:],
                                             start=False, stop=True)
                            nc.vector.tensor_copy(vh[:, nt, 0:HD], psv[:])

                # sgconv matrix load (late: input DMAs get early bandwidth)
                nc.sync.dma_start(
                    sgb[:], sgT_d[:].rearrange("(jt p) i -> p jt i", jt=KT))

                # ---------------- Phase B: attention ----------------
                with tc.tile_pool(name="s_ps", bufs=3, space="PSUM") as s_pool, \
                     tc.tile_pool(name="o_ps", bufs=2, space="PSUM") as o_pool, \
                     tc.tile_pool(name="pt", bufs=3) as pt_pool, \
                     tc.tile_pool(name="os", bufs=2) as os_pool, \
                     tc.tile_pool(name="rc", bufs=2) as rc_pool:

                    def emit_S(qsl, g):
                        ktA, ktB = 2 * g, 2 * g + 1
                        sps = s_pool.tile([128, 1024], dt.float32, tag="sps")
                        nc.tensor.matmul(
                            sps[:, 0:512], T2[0:64, ktA * 128:(ktA + 1) * 128],
                            T1[0:64, qsl], start=True, stop=True,
                            tile_position=(0, 0))
                        nc.tensor.matmul(
                            sps[:, 512:1024],
                            T1[64:128, ktB * 128:(ktB + 1) * 128],
                            T2[64:128, qsl], start=True, stop=True,
                            tile_position=(64, 0), skip_group_check=True)
                        return sps

                    CHUNKS = [[0, 1, 2], [3, 4, 5], [6, 7, 8],
                              [9, 10, 11], [12, 13, 14], [15]]
                    tail = [None]   # deferred (qb, oc) for dbb/recip/mult

                    def emit_tail():
                        if tail[0] is None:
                            return
                        qb, oc = tail[0]
                        tail[0] = None
                        qsl = slice(qb * 512, (qb + 1) * 512)
                        # replicate the denominator row across partitions
                        # 0-63 with a K=1 matmul (ones at array row 64)
                        dbb = o_pool.tile([128, 512], dt.float32, tag="o")
                        nc.tensor.matmul(dbb[0:HD, :], ones_sb[HD:HD + 1, 0:HD],
                                         oc[HD:HD + 1, :], start=True,
                                         stop=True, tile_position=(64, 0),
                                         skip_group_check=True)
                        rc = rc_pool.tile([HD, 512], dt.float32, tag="rc")
                        nc.vector.reciprocal_approx_fast(rc[:], dbb[0:HD, :])
                        nc.vector.tensor_tensor(
                            attnT_sb[:, qsl], oc[0:HD, :], rc[:], Mult)

                    def emit_ag(bounce, gath, n0, n1):
                        nc.sync.dma_start(bounce[:], attnT_sb[:, n0:n1])
                        nc.gpsimd.collective_compute(
                            "AllGather", mybir.AluOpType.bypass,
                            replica_groups=[list(range(NC))],
                            ins=[bounce[:].opt()], outs=[gath[:].opt()])
                        for j in range((n1 - n0) // 128):
                            jt = n0 // 128 + j
                            nc.sync.dma_start_transpose(
                                aj_all[:, jt, :],
                                gath[:, j * 128:(j + 1) * 128])

                    for qb in range(QB):
                        qsl = slice(qb * 512, (qb + 1) * 512)
                        o_ps = o_pool.tile([128, 512], dt.float32, tag="o")
                        store = {g: emit_S(qsl, g) for g in CHUNKS[0]}
                        emit_tail()
                        if qb == 4:
                            emit_ag(bounce_a, gath_a, 0, 2048)
                        elif qb == 7:
                            emit_ag(bounce_b, gath_b, 2048, 3584)
                        for ci, ch in enumerate(CHUNKS):
                            ps_list = []
                            for g in ch:
                                sps = store.pop(g)
                                p = pt_pool.tile([128, 1024], dt.bfloat16,
                                                 tag="pt")
                                if g in ACT_SET:
                                    nc.scalar.activation(p[:], sps[:], Exp)
                                else:
                                    nc.vector.tensor_scalar(
                                        p[:].bitcast(dt.int16), sps[:],
                                        FEXP_A, FEXP_B, Mult, Add)
                                ps_list.append((g, p))
                            if ci + 1 < len(CHUNKS):
                                for g in CHUNKS[ci + 1]:
                                    store[g] = emit_S(qsl, g)
                            for g, p in ps_list:
                                for t in range(2):
                                    kt = 2 * g + t
                                    nc.tensor.matmul(
                                        o_ps[:], vh[:, kt, :],
                                        p[:, t * 512:(t + 1) * 512],
                                        start=(g == 0 and t == 0),
                                        stop=(g == NPAIR - 1 and t == 1),
                                        skip_group_check=True)
                        # free o_ps fast; the rest of the normalize rides
                        # behind the next block's first S-run
                        oc = os_pool.tile([HD + 1, 512], dt.bfloat16, tag="oc")
                        nc.vector.tensor_copy(oc[:], o_ps[0:HD + 1, :])
                        tail[0] = (qb, oc)
                    emit_tail()
                    emit_ag(bounce_c, gath_c, 3584, N)

            # ---------------- Phase C: sgconv (out_sg^T) ----------------
            # gathered attn^T: [512 d, 2048 seq] per half; hardware DMA
            # transpose yields [128 j, 512 d] lhsT tiles directly.
            with tc.tile_pool(name="aj", bufs=4) as aj_pool, \
                 tc.tile_pool(name="og_ps", bufs=1, space="PSUM") as og_pool, \
                 tc.tile_pool(name="pd_sb", bufs=1) as pd_sb_pool:
                og = og_pool.tile([128, NB, RS], dt.float32)
                for jt in range(KT):
                    for db in range(NB):
                        nc.tensor.matmul(
                            og[:, db, :],
                            aj_all[:, jt, db * 128:(db + 1) * 128],
                            sgb[:, jt, :], start=(jt == 0),
                            stop=(jt == KT - 1), skip_group_check=True)
                # ---------------- Phase D: final projection ----------------
                ogT = pd_sb_pool.tile([128, NB, RS], dt.bfloat16)
                for db in range(NB):
                    nc.vector.tensor_copy(ogT[:, db, :], og[:, db, :])
                with tc.tile_pool(name="pd_ps", bufs=2, space="PSUM") as pd_ps_pool, \
                     tc.tile_pool(name="po_sb", bufs=2) as po_sb_pool:
                    for it in range(NB):
                        ps = pd_ps_pool.tile([128, D], dt.float32, tag="pd")
                        for db in range(NB):
                            nc.tensor.matmul(
                                ps[:], ogT[:, db, it * 128:(it + 1) * 128],
                                wo_sb[:, db, :], start=(db == 0), stop=False)
                        nc.tensor.matmul(ps[:], ones_sb[0:1, :], bo_sb[:],
                                         start=False, stop=True)
                        po = po_sb_pool.tile([128, D], dt.float32, tag="po")
                        nc.vector.tensor_copy(po[:], ps[:])
                        nc.sync.dma_start(out_d[it * 128:(it + 1) * 128, :], po[:])
    nc.compile()
    return nc


def kernel(**inputs):
    query = np.asarray(inputs["query"], dtype=np.float32)
    key = np.asarray(inputs["key"], dtype=np.float32)
    value = np.asarray(inputs["value"], dtype=np.float32)
    Wq, bq = np.asarray(inputs["Wq"], np.float32), np.asarray(inputs["bq"], np.float32)
    Wk, bk = np.asarray(inputs["Wk"], np.float32), np.asarray(inputs["bk"], np.float32)
    Wv, bv = np.asarray(inputs["Wv"], np.float32), np.asarray(inputs["bv"], np.float32)
    Wo, bo = np.asarray(inputs["Wo"], np.float32), np.asarray(inputs["bo"], np.float32)
    sg = np.asarray(inputs["sgconv_mat"], np.float32)[0]   # [N, N]

    if "nc" not in _CACHE:
        _CACHE["nc"] = _build()
    nc = _CACHE["nc"]

    qTb = blk(query[0].T.astype(fp8))
    kTb = blk(key[0].T.astype(fp8))
    vTb = blk(value[0].T.astype(bf16))
    wob = blk(Wo.T.astype(bf16))
    common = {
        "qT": qTb, "kT": kTb, "vT": vTb, "wo": wob,
        "bo": bo.reshape(1, D).astype(bf16),
        "ones": np.ones((128, 128), bf16),
        "eye": np.eye(128, dtype=np.float32),
    }
    in_maps = []
    for c in range(NC):
        hs = slice(c * HD, (c + 1) * HD)
        rs = slice(c * RS, (c + 1) * RS)
        in_maps.append(dict(
            common,
            wq=blk((64.0 * SCALE * Wq[hs, :]).T.astype(fp8)),
            wk=blk((64.0 * Wk[hs, :]).T.astype(fp8)),
            wv=blk(Wv[hs, :].T.astype(bf16)),
            bqk=np.concatenate([SCALE * bq[hs], bk[hs]]).reshape(128, 1)
                .astype(np.float32),
            bv=bv[hs].reshape(1, HD).astype(bf16),
            sgT=np.ascontiguousarray(sg[rs, :].T).astype(bf16),
        ))
    res = run_bass_kernel_spmd(nc, in_maps, core_ids=list(range(NC)),
                               **_CACHE.get("run_kwargs", {}))
    _CACHE["last_results"] = res
    out = np.concatenate([res.results[c]["out"] for c in range(NC)], axis=0)
    return out.reshape(1, N, D)
